# revision 1
# baseline (speedup 1.0000x reference)
"""CharEmbLSTMTagger Trainium2 kernel.

Pipeline (per core, all 8 cores run the identical program; host takes core 0):
  A. Load params (host pre-transposed for lhsT layouts).
  B. Per 1024-word block: char-LSTM (transposed layout, one-hot char gathers),
     word_emb indirect-DMA gather, PE transposes -> X^T, big matmul -> per-step
     word-LSTM gate preactivations GX (bias folded), written to DRAM swizzled
     as [chunk, m, p, t'] for the recurrence.
  C. 8192-step sequential word LSTM: For_i over 128 chunks x 64 unrolled steps.
     Per step: 64 accumulating [128,128]x[128,1] matmuls (Whh^T stationary,
     h as 1-column moving operand), gates in PSUM [128,16], elementwise on
     DVE/ACT, h written into ys ring.
  D. Projection + log-softmax per 128-word tile, DMA to output.
"""
import numpy as np
from contextlib import ExitStack

import concourse.bass as bass
import concourse.tile as tile
from concourse import bacc
from concourse import mybir
from concourse.bass import ds
from concourse.bass_utils import run_bass_kernel_spmd
from concourse.masks import make_identity

F32 = mybir.dt.float32
I32 = mybir.dt.int32

W = 8192
LC = 12
CD = 64
ED = 256
HD = 512
CHARSET = 128
VOCAB = 50000
TAGS = 64

BLK = 1024          # phase-B word block
NBLK = W // BLK
U = 64              # recurrence steps per For_i iteration
NIT = W // U


def build_kernel(n_it=NIT, skip_gxout=False, skip_bcast=False, nblk=NBLK, nwt=W//128, nlc=LC, skip_gather=False, skip_gxmm=False):
    nc = bacc.Bacc(None)

    # ---- external params (host-prepared layouts) ----
    p_cidsT = nc.declare_dram_parameter("cidsT", [LC, W], I32, isOutput=False)
    p_sids = nc.declare_dram_parameter("sids2d", [128, W // 128], I32, isOutput=False)
    p_wemb = nc.declare_dram_parameter("word_emb", [W, ED], F32, isOutput=False)
    p_cembT = nc.declare_dram_parameter("cembT", [CD, CHARSET], F32, isOutput=False)
    p_wihcT = nc.declare_dram_parameter("wihcT", [CD, 4 * CD], F32, isOutput=False)
    p_whhcT = nc.declare_dram_parameter("whhcT", [CD, 4 * CD], F32, isOutput=False)
    p_bc = nc.declare_dram_parameter("bc", [1, 4 * CD], F32, isOutput=False)
    p_wihwT = nc.declare_dram_parameter("wihwT", [ED + CD, 4 * HD], F32, isOutput=False)
    p_bw = nc.declare_dram_parameter("bw", [128, 16], F32, isOutput=False)
    p_whhwT = nc.declare_dram_parameter("whhwT", [HD, 4 * HD], F32, isOutput=False)
    p_woutT = nc.declare_dram_parameter("woutT", [HD, TAGS], F32, isOutput=False)
    p_bout = nc.declare_dram_parameter("bout", [1, TAGS], F32, isOutput=False)
    p_iota = nc.declare_dram_parameter("iota128", [128, 1], F32, isOutput=False)
    out_ext = nc.declare_dram_parameter("out", [W, TAGS], F32, isOutput=True)

    with tile.TileContext(nc) as tc, ExitStack() as ctx:
        dram = ctx.enter_context(tc.tile_pool(name="dram", bufs=1, space="DRAM"))
        gx_dram = dram.tile([16, 128, W], F32)           # [m, p, t]
        yst_dram = dram.tile([4, 128, W], F32)           # [k, p, t]

        persist = ctx.enter_context(tc.tile_pool(name="persist", bufs=1))

        # ---- phase A: params to SBUF ----
        cembT = persist.tile([CD, CHARSET], F32)
        nc.sync.dma_start(out=cembT, in_=p_cembT[:])
        wihcT = persist.tile([CD, 4 * CD], F32)
        nc.sync.dma_start(out=wihcT, in_=p_wihcT[:])
        wihw0 = persist.tile([128, 4 * HD], F32)
        nc.sync.dma_start(out=wihw0, in_=p_wihwT[0:128, :])
        wihw1 = persist.tile([128, 4 * HD], F32)
        nc.sync.dma_start(out=wihw1, in_=p_wihwT[128:256, :])
        wihw2 = persist.tile([CD, 4 * HD], F32)
        nc.sync.dma_start(out=wihw2, in_=p_wihwT[256:320, :])
        bw = persist.tile([128, 16], F32)
        nc.sync.dma_start(out=bw, in_=p_bw[:])
        whh = [persist.tile([128, 4 * HD], F32, name=f"whh{k}", tag=f"whh{k}") for k in range(4)]
        for k in range(4):
            nc.sync.dma_start(out=whh[k], in_=p_whhwT[k * 128:(k + 1) * 128, :])
        wout = [persist.tile([128, TAGS], F32, name=f"wout{k}", tag=f"wout{k}") for k in range(4)]
        for k in range(4):
            nc.sync.dma_start(out=wout[k], in_=p_woutT[k * 128:(k + 1) * 128, :])
        bout_b = persist.tile([128, TAGS], F32)
        nc.gpsimd.dma_start(out=bout_b, in_=p_bout[0:1, :].to_broadcast([128, TAGS]))
        iota = persist.tile([128, 1], F32)
        nc.sync.dma_start(out=iota, in_=p_iota[:])
        sids = persist.tile([128, W // 128], I32)
        nc.sync.dma_start(out=sids, in_=p_sids[:])
        ident = persist.tile([128, 128], F32)
        make_identity(nc, ident[:])

        whhcT = persist.tile([CD, 4 * CD], F32)
        nc.sync.dma_start(out=whhcT, in_=p_whhcT[:])
        bc_b = persist.tile([128, 4 * CD], F32)
        nc.gpsimd.dma_start(out=bc_b, in_=p_bc[0:1, :].to_broadcast([128, 4 * CD]))

        # G = char_emb @ Wih_c^T + b_c   [128 charset, 256 gates]
        cembT2 = persist.tile([CD, CHARSET], F32)
        nc.vector.tensor_copy(cembT2[:], cembT[:])
        wihcT2 = persist.tile([CD, 4 * CD], F32)
        nc.vector.tensor_copy(wihcT2[:], wihcT[:])
        with tc.tile_pool(name="gpsum", bufs=1, space="PSUM") as gpsum_pool:
            gpsum = gpsum_pool.tile([CHARSET, 4 * CD], F32)
            nc.tensor.matmul(gpsum[:], lhsT=cembT2[:], rhs=wihcT2[:],
                             start=True, stop=True)
            G = persist.tile([CHARSET, 4 * CD], F32)
            nc.vector.tensor_add(G[:], gpsum[:], bc_b[:])

        # ---- phase B: GX precompute, 8 blocks of 1024 words ----
        with tc.tile_pool(name="pb", bufs=3) as pb, \
             tc.tile_pool(name="pb3", bufs=3) as pb3, \
             tc.tile_pool(name="pbps", bufs=1, space="PSUM") as pbps, \
             tc.tile_pool(name="pbps2", bufs=2, space="PSUM") as pbps2:
            for b in range(nblk):
                hcT = pb.tile([CD, BLK], F32, tag="hcT")
                ccT = pb.tile([CD, BLK], F32, tag="ccT")
                nc.vector.memset(hcT[:], 0.0)
                nc.vector.memset(ccT[:], 0.0)

                for l in range(nlc):
                    cids_lf = pb.tile([CHARSET, BLK], F32, tag="cids_lf")
                    if skip_bcast:
                        nc.vector.memset(cids_lf[:], 0.0)
                    else:
                        cids_li = pb.tile([CHARSET, BLK], I32, tag="cids_li")
                        nc.gpsimd.dma_start(
                            out=cids_li,
                            in_=p_cidsT[l:l + 1, b * BLK:(b + 1) * BLK]
                            .to_broadcast([CHARSET, BLK]))
                        nc.vector.tensor_copy(cids_lf[:], cids_li[:])
                    oh = pb.tile([CHARSET, BLK], F32, tag="oh")
                    nc.vector.tensor_scalar(
                        out=oh[:],
                        in0=cids_lf[:],
                        scalar1=iota[:, 0:1],
                        scalar2=None,
                        op0=mybir.AluOpType.is_equal,
                    )
                    for ni in range(2):
                        sl = slice(ni * 512, (ni + 1) * 512)
                        pgt = []
                        for gi in range(4):  # i, f, g, o gate chunks of 64
                            t = pbps.tile([CD, 512], F32, name=f"pgc{gi}",
                                          tag=f"pgc{gi}")
                            gsl = slice(gi * CD, (gi + 1) * CD)
                            nc.tensor.matmul(
                                t[:], lhsT=G[:, gsl], rhs=oh[:, sl],
                                start=True, stop=False)
                            nc.tensor.matmul(
                                t[:], lhsT=whhcT[:, gsl], rhs=hcT[:, sl],
                                start=False, stop=True)
                            pgt.append(t)
                        si = pb3.tile([CD, 512], F32, tag="si")
                        nc.scalar.activation(si[:], pgt[0][:],
                                             mybir.ActivationFunctionType.Sigmoid)
                        sf = pb3.tile([CD, 512], F32, tag="sf")
                        nc.scalar.activation(sf[:], pgt[1][:],
                                             mybir.ActivationFunctionType.Sigmoid)
                        tg = pb3.tile([CD, 512], F32, tag="tg")
                        nc.scalar.activation(tg[:], pgt[2][:],
                                             mybir.ActivationFunctionType.Tanh)
                        so = pb3.tile([CD, 512], F32, tag="so")
                        nc.scalar.activation(so[:], pgt[3][:],
                                             mybir.ActivationFunctionType.Sigmoid)
                        t1 = pb3.tile([CD, 512], F32, tag="t1")
                        nc.vector.tensor_mul(t1[:], sf[:], ccT[:, sl])
                        t2 = pb3.tile([CD, 512], F32, tag="t2")
                        nc.vector.tensor_mul(t2[:], si[:], tg[:])
                        nc.vector.tensor_add(ccT[:, sl], t1[:], t2[:])
                        tcn = pb3.tile([CD, 512], F32, tag="tcn")
                        nc.scalar.activation(tcn[:], ccT[:, sl],
                                             mybir.ActivationFunctionType.Tanh)
                        nc.vector.tensor_mul(hcT[:, sl], so[:], tcn[:])

                # word embedding gather + transpose into XT0/XT1
                xt0 = pb.tile([128, BLK], F32, tag="xt0")
                xt1 = pb.tile([128, BLK], F32, tag="xt1")
                for j in range(8):
                    wej = pb3.tile([128, ED], F32, tag="wej")
                    if skip_gather:
                        nc.vector.memset(wej[:], 0.1)
                    else:
                        nc.gpsimd.indirect_dma_start(
                            out=wej[:],
                            out_offset=None,
                            in_=p_wemb[:],
                            in_offset=bass.IndirectOffsetOnAxis(
                                ap=sids[:, b * 8 + j:b * 8 + j + 1], axis=0),
                        )
                    for half, xt in ((0, xt0), (1, xt1)):
                        tp = pbps2.tile([128, 128], F32, tag="tp")
                        nc.tensor.transpose(
                            tp[:], wej[:, half * 128:(half + 1) * 128], ident[:])
                        nc.vector.tensor_copy(xt[:, j * 128:(j + 1) * 128], tp[:])

                # GX^T = Wih_w^T.T @ X^T + b  -> swizzled DRAM
                for m in range(16 if not skip_gxmm else 0):
                    for ni in range(2):
                        pgx = pbps2.tile([128, 512], F32, tag="pgx")
                        msl = slice(m * 128, (m + 1) * 128)
                        nsl = slice(ni * 512, (ni + 1) * 512)
                        nc.tensor.matmul(pgx[:], lhsT=wihw0[:, msl],
                                         rhs=xt0[:, nsl], start=True, stop=False)
                        nc.tensor.matmul(pgx[:], lhsT=wihw1[:, msl],
                                         rhs=xt1[:, nsl], start=False, stop=False)
                        nc.tensor.matmul(pgx[:], lhsT=wihw2[:, msl],
                                         rhs=hcT[:, nsl], start=False, stop=True)
                        gxs = pb3.tile([128, 512], F32, tag="gxs")
                        nc.vector.tensor_scalar_add(gxs[:], pgx[:], bw[:, m:m + 1])
                        # 512 step-cols = 8 chunks x 64
                        if not skip_gxout:
                            t0 = b * BLK + ni * 512
                            mc = m if m < 8 else (m + 4 if m < 12 else m - 4)
                            nc.sync.dma_start(
                                out=gx_dram[mc, :, t0:t0 + 512], in_=gxs[:])

        # ---- phase C: sequential word LSTM ----
        h_prev = persist.tile([128, 4], F32)
        c_st = persist.tile([128, 4], F32)
        nc.vector.memset(h_prev[:], 0.0)
        nc.vector.memset(c_st[:], 0.0)

        with tc.tile_pool(name="pc", bufs=2) as pc, \
             tc.tile_pool(name="pc3", bufs=3) as pc3, \
             tc.tile_pool(name="pcps", bufs=2, space="PSUM") as pcps:
            with tc.For_i(0, n_it, 1, staggered_reset=True, hint_engines=(
                    mybir.EngineType.PE, mybir.EngineType.DVE)) as it:
                gxt = pc.tile([128, 16, U], F32, tag="gxt")
                src = gx_dram[:, :, ds(it * U, U)].rearrange("m p t -> p m t")
                nc.sync.dma_start(out=gxt[:], in_=src)
                ys = pc.tile([128, 4 * U], F32, tag="ys")
                ys3 = ys.rearrange("p (k t) -> p t k", k=4)
                for t in range(U):
                    rhs = None if t == 0 else ys

                    pg = pcps.tile([128, 16], F32, tag="pgr")
                    for m in range(16):
                        mc = m if m < 8 else (m + 4 if m < 12 else m - 4)
                        for k in range(4):
                            rk = (h_prev[:, k:k + 1] if t == 0
                                  else ys[:, k * U + t - 1:k * U + t])
                            nc.tensor.matmul(
                                pg[:, mc:mc + 1],
                                lhsT=whh[k][:, m * 128:(m + 1) * 128],
                                rhs=rk,
                                start=(k == 0), stop=(k == 3))
                    gsb = pc3.tile([128, 16], F32, tag="gsb")
                    nc.vector.tensor_add(gsb[:], pg[:], gxt[:, :, t])
                    sif = pc3.tile([128, 12], F32, tag="sifr")
                    nc.scalar.activation(sif[:], gsb[:, 0:12],
                                         mybir.ActivationFunctionType.Sigmoid)
                    tg = pc3.tile([128, 4], F32, tag="tgr")
                    nc.scalar.activation(tg[:], gsb[:, 12:16],
                                         mybir.ActivationFunctionType.Tanh)
                    t1 = pc3.tile([128, 4], F32, tag="t1r")
                    nc.vector.tensor_mul(t1[:], sif[:, 4:8], c_st[:])
                    t2 = pc3.tile([128, 4], F32, tag="t2r")
                    nc.vector.tensor_mul(t2[:], sif[:, 0:4], tg[:])
                    nc.vector.tensor_add(c_st[:], t1[:], t2[:])
                    tcn = pc3.tile([128, 4], F32, tag="tcnr")
                    nc.scalar.activation(tcn[:], c_st[:],
                                         mybir.ActivationFunctionType.Tanh)
                    nc.vector.tensor_mul(ys3[:, t, :], sif[:, 8:12], tcn[:])
                nc.vector.tensor_copy(h_prev[:], ys3[:, U - 1, :])
                ydst = yst_dram[:, :, ds(it * U, U)].rearrange("k p t -> p k t")
                ysrc = ys.rearrange("p (k t) -> p k t", k=4)
                nc.sync.dma_start(out=ydst, in_=ysrc)

        # ---- phase D: projection + log_softmax ----
        with tc.tile_pool(name="pd", bufs=3) as pd, \
             tc.tile_pool(name="pdps", bufs=2, space="PSUM") as pdps:
            for wt in range(nwt):
                yt = pd.tile([128, 512], F32, tag="yt")
                ysrc2 = yst_dram[:, :, wt * 128:(wt + 1) * 128].rearrange(
                    "k p t -> p k t")
                nc.sync.dma_start(out=yt, in_=ysrc2)
                pl = pdps.tile([128, TAGS], F32, tag="pl")
                for k in range(4):
                    nc.tensor.matmul(pl[:],
                                     lhsT=yt[:, k * 128:(k + 1) * 128],
                                     rhs=wout[k][:],
                                     start=(k == 0), stop=(k == 3))
                lg = pd.tile([128, TAGS], F32, tag="lg")
                nc.vector.tensor_add(lg[:], pl[:], bout_b[:])
                mx = pd.tile([128, 1], F32, tag="mx")
                nc.vector.tensor_reduce(mx[:], lg[:], axis=mybir.AxisListType.X,
                                        op=mybir.AluOpType.max)
                lgs = pd.tile([128, TAGS], F32, tag="lgs")
                nc.vector.tensor_scalar_sub(lgs[:], lg[:], mx[:, 0:1])
                ex = pd.tile([128, TAGS], F32, tag="ex")
                se = pd.tile([128, 1], F32, tag="se")
                nc.scalar.activation(ex[:], lgs[:],
                                     mybir.ActivationFunctionType.Exp,
                                     accum_out=se[:, 0:1])
                lns = pd.tile([128, 1], F32, tag="lns")
                nc.scalar.activation(lns[:], se[:],
                                     mybir.ActivationFunctionType.Ln)
                ot = pd.tile([128, TAGS], F32, tag="ot")
                nc.vector.tensor_scalar_sub(ot[:], lgs[:], lns[:, 0:1])
                nc.sync.dma_start(out=out_ext[wt * 128:(wt + 1) * 128, :], in_=ot[:])

    nc.finalize()
    return nc


_NC_CACHE = None
_LAST_IN_MAP = None


def kernel(**inputs):
    global _NC_CACHE, _LAST_IN_MAP
    cs = np.ascontiguousarray(np.asarray(inputs["char_sentence"], np.int32))
    sent = np.asarray(inputs["sentence"], np.int32)
    # prune word_emb to the <=W unique rows actually gathered; remap indices
    wemb_full = np.asarray(inputs["word_emb"], np.float32)
    uniq, inv = np.unique(sent, return_inverse=True)
    wtab = np.zeros((W, ED), np.float32)
    wtab[:len(uniq)] = wemb_full[uniq]
    sent = inv.astype(np.int32)
    in_map = {
        "cidsT": np.ascontiguousarray(cs.T),
        "sids2d": np.ascontiguousarray(sent.reshape(W // 128, 128).T),
        "word_emb": wtab,
        "cembT": np.ascontiguousarray(np.asarray(inputs["char_emb"], np.float32).T),
        "wihcT": np.ascontiguousarray(np.asarray(inputs["Wih_c"], np.float32).T),
        "whhcT": np.ascontiguousarray(np.asarray(inputs["Whh_c"], np.float32).T),
        "bc": (np.asarray(inputs["bih_c"], np.float32)
               + np.asarray(inputs["bhh_c"], np.float32)).reshape(1, -1),
        "wihwT": np.ascontiguousarray(np.asarray(inputs["Wih_w"], np.float32).T),
        "bw": np.ascontiguousarray(
            (np.asarray(inputs["bih_w"], np.float32)
             + np.asarray(inputs["bhh_w"], np.float32)).reshape(16, 128).T),
        "whhwT": np.ascontiguousarray(np.asarray(inputs["Whh_w"], np.float32).T),
        "woutT": np.ascontiguousarray(np.asarray(inputs["W_out"], np.float32).T),
        "bout": np.asarray(inputs["b_out"], np.float32).reshape(1, -1),
        "iota128": np.arange(128, dtype=np.float32).reshape(128, 1),
    }
    _LAST_IN_MAP = in_map
    if _NC_CACHE is None:
        _NC_CACHE = build_kernel()
    res = run_bass_kernel_spmd(_NC_CACHE, [in_map] * 8, list(range(8)))
    return np.asarray(res.results[0]["out"], np.float32)


if __name__ == "__main__":
    import reference
    inp = reference.setup_inputs()
    out = kernel(**{k: np.asarray(v) for k, v in inp.items()})
    print(out.shape, out.dtype)

